# revision 9
# baseline (speedup 1.0000x reference)
"""Trainium2 Bass kernel for nn_Conv2dAMPS.

Reference computation: im2col with a 2x2 kernel (4 positions), per-sample
matrices M_w = tensors[w] . emb_w (contract channels), output = row 0 of
M_0 @ M_1 @ M_2 @ M_3, reshaped to (B, O, oh, ow).

Only row 0 of the matrix product is needed, so the chain collapses to a
vector-matrix chain per sample.  Additionally the first two links are
merged into one bilinear stage via the precomputed
    U[(p0,p1), j] = sum_i T0[0,i,p0] * T1[i,j,p1]
so the whole computation is three kron-contraction stages:
    A:  v1 = kron(e0, e1) @ U
    B:  v2 = kron(v1, e2) @ W2r      (W2r[(i,p),j] = T2[i,j,p])
    C:  out = kron(v2, e3) @ W3r
Each stage: the per-sample outer product z = a (x) b is built on the
vector engine (plus a few decoupled chunks on gpsimd) from
partition-replicated operand tiles produced by 0/1-selection matmuls on
the tensor engine; z chunks (8 a's x 16 b's = 128 lanes) feed K=128
matmuls accumulating into PSUM with 2x column tiling (even/odd chunks in
top/bottom array halves), and the top+bottom fold is absorbed into the
next stage's replication matmul (K=128 selection).

Stage A has no dependency on previous-stage results, so its operand
replication can begin as soon as the (host-side packed, contiguous)
im2col loads land -- there is no serial "v0 + pats" head.  pats for
stage k+1 are built during stage k.

im2col (host-side packing): e0ext = x[:, 0:32, 0:31] (n = h*31+w),
e1ext = x[:, 0:32, 1:32]; e2 = e0ext[n+31], e3 = e1ext[n+31], so the
kernel only ever slices one SBUF-resident pair of extended images.

Sharding: data-parallel over batch B (2 images per core, 8 cores),
weights replicated.
"""

import sys

sys.path.insert(0, "/opt/trn_rl_repo")

import numpy as np
import ml_dtypes

import concourse.bacc as bacc
import concourse.mybir as mybir
import concourse.tile as tile
from concourse import bass_utils

BF16 = ml_dtypes.bfloat16

B, C, H, W = 16, 64, 32, 32
O = 64
OH = OW = 31
NIMG = OH * OW            # 961 real samples per image
NEXT = 992                # extended im2col columns (32 rows x 31)
BLK = 1024                # column block per image
NCORES = 8
IPC = B // NCORES         # images per core
FD = IPC * BLK            # 2048 free columns per core
NQ = FD // 512            # psum quarters

A_SET = 8                 # left-factor values per chunk-set (replicated 16x)
B_SET = 16                # right-factor values per chunk-set (replicated 8x)
N_S = O // A_SET          # 8
N_T = O // B_SET          # 4
N_CHUNK = N_S * N_T       # 32

# (right src slot, right shift) per stage; the left factor of stage A is
# e0 (slot 0 shift 0); stages B/C take the left factor from the previous
# stage's v.
STAGE_RIGHT = {0: (1, 0), 1: (0, 31), 2: (1, 31)}

# chunks (by (s,t) index c = s*N_T+t) assigned to gpsimd per stage.
# gpsimd TT is ~4x slower than DVE; give it a few chunks that are
# consumed late in the accumulation order so its latency stays hidden.
GP_CHUNKS = {0: (), 1: (), 2: ()}
# number of DVE chunks kept after the last gpsimd chunk in the
# accumulation order (so PE never stalls on a late gpsimd TT).
N_DVE_TAIL = 4


def _chunk_order(stage):
    gp = GP_CHUNKS[stage]
    dve = [c for c in range(N_CHUNK) if c not in gp]
    head = dve[: len(dve) - N_DVE_TAIL]
    tail = dve[len(dve) - N_DVE_TAIL:]
    order = head + list(gp) + tail
    return order, set(gp)


def _build_program(reps=1, loop_n=1):
    nc = bacc.Bacc("TRN2", target_bir_lowering=False, debug=False)
    dt = mybir.dt

    # host-packed extended im2col: (IPC, C, 2, BLK) -- e0ext / e1ext
    xe_d = nc.dram_tensor("xe", [IPC, C, 2, BLK], dt.bfloat16,
                          kind="ExternalInput").ap()
    # stage weights, partition-major: [128, 3, N_CHUNK, O]
    lhst_d = nc.dram_tensor("lhst", [128, 3, N_CHUNK, O], dt.bfloat16,
                            kind="ExternalInput").ap()
    # stage-A left replication (K=64, no fold)
    ra_d = nc.dram_tensor("ra", [C, N_S, 128], dt.bfloat16,
                          kind="ExternalInput").ap()
    # stage-B/C left replication (K=128, folds top+bottom acc halves)
    r1_d = nc.dram_tensor("r1", [128, N_S, 128], dt.bfloat16,
                          kind="ExternalInput").ap()
    # right replication (K=64)
    r2_d = nc.dram_tensor("r2", [C, N_T, 128], dt.bfloat16,
                          kind="ExternalInput").ap()
    out_d = nc.dram_tensor("out", [IPC, O, NIMG], dt.float32,
                           kind="ExternalOutput").ap()

    with tile.TileContext(nc) as tc:
        with (
            tc.tile_pool(name="consts", bufs=1) as consts,
            tc.tile_pool(name="embp", bufs=1) as embp,
            tc.tile_pool(name="patp", bufs=2) as patp,
            tc.tile_pool(name="ops1", bufs=2) as ops1,
            tc.tile_pool(name="zp", bufs=3) as zp,
            tc.tile_pool(name="vp", bufs=2) as vp,
            tc.tile_pool(name="outp", bufs=1) as outp,
            tc.tile_pool(name="ps_op", bufs=2, space="PSUM") as ps_op,
            tc.tile_pool(name="ps_acc", bufs=1, space="PSUM") as ps_acc,
        ):
            # ---- input image first (unblocks warmup + stage-A operands),
            # then selection matrices, then the big stage weights.
            embT = embp.tile([C, 2, FD + 32], dt.bfloat16)
            nc.vector.memset(embT[:, :, FD:FD + 32], 0.0)
            for b in range(IPC):
                for e in range(2):
                    nc.sync.dma_start(
                        out=embT[:, e, b * BLK:(b + 1) * BLK],
                        in_=xe_d[b, :, e, :])
            ra_sb = consts.tile([C, N_S, 128], dt.bfloat16)
            nc.sync.dma_start(out=ra_sb, in_=ra_d)
            r2_sb = consts.tile([C, N_T, 128], dt.bfloat16)
            nc.sync.dma_start(out=r2_sb, in_=r2_d)
            r1_sb = consts.tile([128, N_S, 128], dt.bfloat16)
            nc.sync.dma_start(out=r1_sb, in_=r1_d)
            lhst_sb = consts.tile([128, 3, N_CHUNK, O], dt.bfloat16)
            nc.sync.dma_start(out=lhst_sb, in_=lhst_d)

            # ---- PE warmup: dense tiny matmuls flip the HAM clock gate to
            # 8/8 while the weight DMAs stream; gated only on the first xe
            # transfer.  Output lands in embT's zero tail (junk columns).
            wps = ps_op.tile([128, 512], dt.float32, tag="op", name="warm_ps")
            for w in range(72):
                nc.tensor.matmul(wps[:, (w % 4) * 128:(w % 4) * 128 + 128],
                                 embT[:, 0, 0:128],
                                 embT[:, 0, 128:256],
                                 start=True, stop=True)
            nc.scalar.copy(out=embT[0:C, 1, FD + 16:FD + 32],
                           in_=wps[0:C, 0:16])

            loop_cm = tc.For_i(0, loop_n, 1) if loop_n > 1 else None
            import contextlib
            with (loop_cm if loop_cm is not None else contextlib.nullcontext()):
                for rep in range(reps):
                    def pat_slice(pat, b, stage, t):
                        """one replicated right-factor t-slice for image b."""
                        e, off = STAGE_RIGHT[stage]
                        base = b * BLK + off
                        p2 = ps_op.tile([128, 1024], dt.float32, tag="op",
                                        name=f"patp_{rep}_{b}_{stage}_{t}")
                        for q in range(2):
                            c0 = base + q * 512
                            nc.tensor.matmul(p2[:, q * 512:(q + 1) * 512],
                                             r2_sb[:, t, :],
                                             embT[:, e, c0:c0 + 512],
                                             start=True, stop=True)
                        nc.scalar.copy(out=pat[:, t, :], in_=p2)

                    def op1_slice(op1, b, stage, s, vT):
                        """one replicated left-factor s-slice for image b."""
                        p1 = ps_op.tile([128, 1024], dt.float32, tag="op",
                                        name=f"op1p_{rep}_{b}_{stage}_{s}")
                        for q in range(2):
                            if stage == 0:
                                nc.tensor.matmul(p1[:, q * 512:(q + 1) * 512],
                                                 ra_sb[:, s, :],
                                                 embT[:, 0, b * BLK + q * 512:
                                                      b * BLK + q * 512 + 512],
                                                 start=True, stop=True)
                            else:
                                nc.tensor.matmul(p1[:, q * 512:(q + 1) * 512],
                                                 r1_sb[:, s, :],
                                                 vT[:, q * 512:(q + 1) * 512],
                                                 start=True, stop=True)
                        nc.scalar.copy(out=op1[:, s, :], in_=p1)

                    # per-image stage pipeline state
                    op1s = [None, None]
                    pats = [None, None]

                    # stage A operands; interleave images so img0's first
                    # quad unblocks quickly and img1 follows.
                    for b in range(IPC):
                        op1s[b] = ops1.tile([128, N_S, BLK], dt.bfloat16,
                                            tag=f"op1_{b}", name=f"op1_{rep}_0_{b}")
                        pats[b] = patp.tile([128, N_T, BLK], dt.bfloat16,
                                            tag=f"pat_{b}", name=f"pat_{rep}_0_{b}")
                    op1_slice(op1s[0], 0, 0, 0, None)
                    for t in range(N_T):
                        pat_slice(pats[0], 0, 0, t)
                    for s in range(1, N_S):
                        op1_slice(op1s[0], 0, 0, s, None)
                    op1_slice(op1s[1], 1, 0, 0, None)
                    for t in range(N_T):
                        pat_slice(pats[1], 1, 0, t)
                    for s in range(1, N_S):
                        op1_slice(op1s[1], 1, 0, s, None)

                    for stage in range(3):
                        for b in range(IPC):
                            op1, pat = op1s[b], pats[b]
                            acc = [ps_acc.tile([128, 512], dt.float32,
                                               tag=f"acc_{b}_{q}",
                                               name=f"acc_{rep}_{stage}_{b}_{q}")
                                   for q in range(2)]
                            for s in range(N_S):
                                z = zp.tile([128, N_T, BLK], dt.bfloat16,
                                            tag=f"z_{b}",
                                            name=f"z_{rep}_{stage}_{b}_{s}")
                                nc.vector.tensor_mul(
                                    z,
                                    op1[:, s, :].rearrange(
                                        "p (x n) -> p x n", x=1
                                    ).broadcast_to([128, N_T, BLK]),
                                    pat)
                                for t in range(N_T):
                                    half = t % 2
                                    tp = (0, 64 * half)
                                    row = slice(64 * half, 64 * (half + 1))
                                    for q in range(2):
                                        nc.tensor.matmul(
                                            acc[q][row, :],
                                            lhst_sb[:, stage, s * N_T + t, :],
                                            z[:, t, q * 512:(q + 1) * 512],
                                            start=(s == 0 and t < 2),
                                            stop=(s == N_S - 1 and t >= 2),
                                            tile_position=tp)
                                # build next stage's right factor mid-stage
                                if stage < 2 and 1 <= s <= N_T:
                                    if s == 1:
                                        pats[b] = patp.tile(
                                            [128, N_T, BLK], dt.bfloat16,
                                            tag=f"pat_{b}",
                                            name=f"pat_{rep}_{stage + 1}_{b}")
                                    pat_slice(pats[b], b, stage + 1, s - 1)
                            if stage < 2:
                                vT = vp.tile([128, BLK], dt.bfloat16,
                                             tag=f"v_{b}",
                                             name=f"v_{rep}_{stage}_{b}")
                                op1n = ops1.tile([128, N_S, BLK], dt.bfloat16,
                                                 tag=f"op1_{b}",
                                                 name=f"op1_{rep}_{stage + 1}_{b}")
                                for q in range(2):
                                    nc.scalar.copy(
                                        out=vT[:, q * 512:(q + 1) * 512],
                                        in_=acc[q])
                                for s in range(N_S):
                                    op1_slice(op1n, b, stage + 1, s, vT)
                                op1s[b] = op1n
                            else:
                                vtop = outp.tile([O, BLK], dt.float32,
                                                 tag=f"vtop_{b}",
                                                 name=f"vtop_{rep}_{b}")
                                outT = outp.tile([O, BLK], dt.float32,
                                                 tag=f"outT_{b}",
                                                 name=f"outT_{rep}_{b}")
                                for q in range(2):
                                    sl = slice(q * 512, (q + 1) * 512)
                                    nc.scalar.copy(out=vtop[:, sl],
                                                   in_=acc[q][0:O, :])
                                    nc.vector.tensor_add(outT[:, sl],
                                                         vtop[:, sl],
                                                         acc[q][O:128, :])
                                nc.sync.dma_start(out=out_d[b],
                                                  in_=outT[:, 0:NIMG])

    nc.compile()
    return nc


def _build_weights(tensors):
    T = np.asarray(tensors, dtype=np.float32)  # (4, O, O, C): [w, i, j, p]
    lhst = np.zeros((3, N_CHUNK, 128, O), dtype=BF16)
    # stage A: U[(p0,p1), j] = sum_i T0[0,i,p0] * T1[i,j,p1]
    U = np.einsum('ip,ijq->pqj', T[0, 0], T[1])          # (p0, p1, j)
    for s in range(N_S):
        for t in range(N_T):
            blk = U[s * A_SET:(s + 1) * A_SET,
                    t * B_SET:(t + 1) * B_SET, :]
            lhst[0, s * N_T + t] = blk.reshape(128, O).astype(BF16)
    # stages B/C: W[(i,p), j] = T[k][i, j, p]
    for k in (2, 3):
        t_ipj = np.ascontiguousarray(T[k].transpose(0, 2, 1))    # (i, p, j)
        for s in range(N_S):
            for t in range(N_T):
                blk = t_ipj[s * A_SET:(s + 1) * A_SET,
                            t * B_SET:(t + 1) * B_SET, :]
                lhst[k - 1, s * N_T + t] = blk.reshape(128, O).astype(BF16)
    # stage-A left selection: lane <- e0 row s*8 + lane//16  (K=64)
    ra = np.zeros((C, N_S, 128), dtype=BF16)
    for s in range(N_S):
        for lane in range(128):
            ra[s * A_SET + lane // B_SET, s, lane] = 1.0
    # stage-B/C left selection with fold (K=128)
    r1 = np.zeros((128, N_S, 128), dtype=BF16)
    for s in range(N_S):
        for lane in range(128):
            i = s * A_SET + lane // B_SET
            r1[i, s, lane] = 1.0
            r1[O + i, s, lane] = 1.0
    # right selection: lane <- row t*16 + lane%16  (K=64)
    r2 = np.zeros((C, N_T, 128), dtype=BF16)
    for t in range(N_T):
        for lane in range(128):
            r2[t * B_SET + lane % B_SET, t, lane] = 1.0
    # lhst partition-major: (128, 3, N_CHUNK, O)
    lhst = np.ascontiguousarray(lhst.transpose(2, 0, 1, 3))
    return {"lhst": lhst, "ra": ra, "r1": r1, "r2": r2}


def _pack_inputs(input_data):
    """host-side im2col packing: (B, C, 2, BLK) bf16, n = h*31+w."""
    x = np.asarray(input_data, dtype=np.float32)
    xe = np.zeros((B, C, 2, BLK), dtype=BF16)
    xe[:, :, 0, :NEXT] = x[:, :, :, 0:31].reshape(B, C, NEXT).astype(BF16)
    xe[:, :, 1, :NEXT] = x[:, :, :, 1:32].reshape(B, C, NEXT).astype(BF16)
    return xe


_CACHE = {}


def _get_program(reps=1, loop_n=1):
    key = f"nc{reps}_{loop_n}"
    if key not in _CACHE:
        _CACHE[key] = _build_program(reps, loop_n)
    return _CACHE[key]


def run(input_data, tensors, trace=False, reps=1, loop_n=1):
    nc = _get_program(reps, loop_n)
    w = _build_weights(tensors)
    xe = _pack_inputs(input_data)
    in_maps = []
    for c in range(NCORES):
        m = dict(w)
        m["xe"] = np.ascontiguousarray(xe[c * IPC:(c + 1) * IPC])
        in_maps.append(m)
    res = bass_utils.run_bass_kernel_spmd(nc, in_maps, core_ids=list(range(NCORES)),
                                          trace=trace)
    outs = np.concatenate([res.results[c]["out"] for c in range(NCORES)], axis=0)
    out = outs.reshape(B, O, OH, OW).astype(np.float32)
    return out, res


def kernel(input_data, tensors):
    out, _ = run(input_data, tensors)
    return out
